# revision 31
# baseline (speedup 1.0000x reference)
"""Trainium2 Bass kernel for FeatureEmbedding (embedding_lookup).

Computes, for inputs x[B,F], mask_positions[B,NM], W[F,D], b[F,D], mask_token[D]:

    tokens[bb, f, :] = x[bb, f] * W[f, :] + b[f, :]
    tokens[bb, p, :] = mask_token          for p in mask_positions[bb]

Strategy (data-parallel over 8 NeuronCores, 256 batch rows each):
  - On device, build a one-hot mask m[bb,f] = any_j(mask_positions[bb,j]==f)
    with iota/is_equal vector ops, then om = 1-m and xs = x*om.
  - For each feature f the output tile out[:, f, :] (128 batch rows x 512) is
    a rank-3 product computed on the TensorEngine:
        out = xs[:,f] (x) W[f,:] + om[:,f] (x) b[f,:] + m[:,f] (x) mask_token
    i.e. a K=3 matmul with stationary rows (xs_f, om_f, m_f) and moving rows
    (W[f], b[f], mask_token).  The stationary triples are produced by PE
    transposes of a packed tile; four features run concurrently in the four
    32-row PE tile groups (row tiling).  PSUM tiles are evicted to SBUF by
    the Vector and Scalar engines (split), then streamed to HBM in 2 MiB DMAs.
  - Matmul operands use float32r (same 4-byte layout as f32; fast PE path).

The output write (32 MiB/core, 256 MiB total) is the roofline term.
"""

import dataclasses
import os
from contextlib import ExitStack

import numpy as np

import concourse.bass as bass
import concourse.tile as tile
from concourse import bacc, mybir
from concourse.bass_utils import run_bass_kernel_spmd

B, F, D, NM = 2048, 64, 512, 8
NCORES = 8
B_LOC = B // NCORES          # 256 batch rows per core
NT = B_LOC // 128            # 2 partition-tiles of 128 batch rows
NI = F // 4                  # 16 stationary packs (4 features each)

F32 = mybir.dt.float32
# Output chunking: feature spans per output DMA (small first for fast start).
CHUNKS = [(0, 2), (2, 2), (4, 4)] + [(8 * k, 8) for k in range(1, 8)]
# float32r: same bits as f32, streams through the PE at 1 cycle/row
# (fp32 proper needs 4).  Flip to False if precision ever becomes an issue.
USE_F32R = os.environ.get("KERNEL_NO_F32R", "") == ""
FMM = mybir.dt.float32r if USE_F32R else mybir.dt.float32
AOT = mybir.AluOpType

_cache: dict = {}


def _body(ctx: ExitStack, tc: tile.TileContext, x_ap, mp_ap, r_ap, id_ap,
          io_ap, out_ap):
    nc = tc.nc
    singles = ctx.enter_context(tc.tile_pool(name="singles", bufs=1))
    eqpool = ctx.enter_context(tc.tile_pool(name="eqp", bufs=2))
    spool = ctx.enter_context(tc.tile_pool(name="spool", bufs=20))
    stpool = ctx.enter_context(tc.tile_pool(name="stage", bufs=4))
    tpsum = ctx.enter_context(tc.tile_pool(name="tpsum", bufs=4, space="PSUM"))
    opsum = ctx.enter_context(tc.tile_pool(name="opsum", bufs=4, space="PSUM"))

    # All input DMAs issue up-front on the HWDGE (sync) path: the batch-tile
    # inputs first (they gate the first output chunk), then params/constants.
    # Input DMAs on the sync HWDGE ring, ordered by when the pipeline needs
    # them: the mask chain (mp, iota, x) gates everything else.
    xts = [singles.tile([128, F], F32, tag=f"xt{t}", name=f"xt{t}")
           for t in range(NT)]
    mpts = [singles.tile([128, NM], F32, tag=f"mpt{t}", name=f"mpt{t}")
            for t in range(NT)]
    iota = singles.tile([128, F], F32, tag="iota")
    ident = singles.tile([128, 128], F32, tag="ident")
    nc.sync.dma_start(out=mpts[0][:], in_=mp_ap[0:128, :])
    nc.sync.dma_start(out=iota[:], in_=io_ap[:, :])
    nc.sync.dma_start(out=xts[0][:], in_=x_ap[0:128, :])
    nc.sync.dma_start(out=ident[:], in_=id_ap[:, :])
    for t in range(1, NT):
        nc.sync.dma_start(out=mpts[t][:], in_=mp_ap[t * 128:(t + 1) * 128, :])
        nc.sync.dma_start(out=xts[t][:], in_=x_ap[t * 128:(t + 1) * 128, :])

    # R holds the moving operands: PE row-group q serves features f = q mod 4;
    # partitions 32q+{0,1,2} hold (W[f], b[f], mask_token) of f = 4i+q at
    # free cols i*D.  The 3-partition transfers are slow, so they ride the
    # scalar-engine HWDGE ring (separate FIFO from the output ring) as a
    # column ladder: the first four small DMAs unlock output chunk 0, the
    # tails stream in behind the early chunks.
    R = singles.tile([128, NI * D], FMM, tag="R")
    for q in range(4):
        nc.scalar.dma_start(out=R[32 * q:32 * q + 3, 0:4 * D],
                            in_=r_ap[3 * q:3 * q + 3, 0:4 * D])

    def emit_r_tails():
        # Issued mid-stream so the scalar engine runs the first output-chunk
        # evictions before spending queue time on these triggers.
        for q in range(4):
            nc.scalar.dma_start(out=R[32 * q:32 * q + 3, 4 * D:],
                                in_=r_ap[3 * q:3 * q + 3, 4 * D:])

    # P (one per batch-tile, ping-pong) packs the per-batch stationary data in
    # natural layout so one PE transpose per 128 columns yields the stationary
    # tile: column 128i + 32g + c = component c (xs, om, m) of feature 4i+g.
    # Columns 32g+3..32g+31 are never written: they land in stationary rows
    # the K=3 matmuls never read, so no zeroing is needed.
    Ps = [singles.tile([128, NI * 128], F32, tag=f"P{t}", name=f"P{t}")
          for t in range(NT)]

    # Feature map: f = 4i + g -> stationary pack S_i, PE row-group g.
    iov = iota[:, :].rearrange("p (i g) -> p i g", g=4)

    for T in range(NT):
        xt = xts[T]
        mpt = mpts[T]
        P = Ps[T]
        Pv = P[:, :].rearrange("p (i g c) -> p i g c", g=4, c=32)
        xsv = Pv[:, :, :, 0]
        omv = Pv[:, :, :, 1]
        mv = Pv[:, :, :, 2]
        xv = xt[:, :].rearrange("p (i g) -> p i g", g=4)

        # m = one-hot of mask positions: compare iota against every index
        # column in one broadcast op, then max-reduce over the index axis.
        eq = eqpool.tile([128, F * NM], F32, tag="eq")
        eqv = eq[:, :].rearrange("p (i g j) -> p i g j", g=4, j=NM)
        iov4 = iov.to_broadcast([128, NI, 4, NM])
        mpv4 = dataclasses.replace(
            mpt[:, :], ap=[[NM, 128], [0, NI], [0, 4], [1, NM]])
        nc.vector.tensor_tensor(eqv, iov4, mpv4, op=AOT.is_equal)
        nc.vector.tensor_reduce(mv, eqv, axis=mybir.AxisListType.X,
                                op=AOT.max)
        nc.vector.tensor_scalar(omv, mv, -1.0, 1.0, AOT.mult, AOT.add)
        nc.vector.tensor_tensor(xsv, xv, omv, op=AOT.mult)

        # Transpose the 16 packs into stationary tiles.
        Ss = []
        for i in range(NI):
            pst = tpsum.tile([128, 128], F32, tag="pst")
            nc.tensor.transpose(pst[:], P[:, 128 * i:128 * (i + 1)], ident[:])
            S = spool.tile([128, 128], FMM, tag="S")
            if i % 2 == 0:
                nc.vector.tensor_copy(S[:], pst[:])
            else:
                nc.scalar.copy(S[:], pst[:])
            Ss.append(S)

        # 64 K=3 row-tiled matmuls; evict PSUM->SBUF stage; DMA out per
        # chunk.  The first chunks are small so the output stream starts as
        # early as possible.
        for ci, (s0, nf) in enumerate(CHUNKS):
            stg = stpool.tile([128, 8 * D], F32, tag="stg")
            for k in range(nf):
                f = s0 + k
                i, g = divmod(f, 4)
                po = opsum.tile([128, D], F32, tag="po")
                nc.tensor.matmul(po[:], lhsT=Ss[i][32 * g:32 * g + 3, :],
                                 rhs=R[32 * g:32 * g + 3, bass.ts(i, D)],
                                 start=True, stop=True,
                                 tile_position=(32 * g, 0))
                if f % 2 == 0:
                    nc.vector.tensor_copy(stg[:, bass.ts(k, D)], po[:])
                else:
                    nc.scalar.copy(stg[:, bass.ts(k, D)], po[:])
            nc.sync.dma_start(
                out=out_ap[T * 128:(T + 1) * 128, s0:s0 + nf, :],
                in_=stg[:, 0:nf * D].rearrange("p (a b) -> p a b", b=D))
            if T == 0 and ci == 1:
                emit_r_tails()


def _build_program():
    nc = bacc.Bacc("TRN2", target_bir_lowering=False, debug=False,
                   enable_asserts=True, num_devices=NCORES)
    x_d = nc.dram_tensor("x_in", [B_LOC, F], F32, kind="ExternalInput")
    mp_d = nc.dram_tensor("mp_in", [B_LOC, NM], F32, kind="ExternalInput")
    r_d = nc.dram_tensor("r_in", [12, NI * D], FMM, kind="ExternalInput")
    id_d = nc.dram_tensor("id_in", [128, 128], F32, kind="ExternalInput")
    io_d = nc.dram_tensor("io_in", [128, F], F32, kind="ExternalInput")
    out_d = nc.dram_tensor("out", [B_LOC, F, D], F32, kind="ExternalOutput")

    with tile.TileContext(nc) as tc:
        with ExitStack() as ctx:
            _body(ctx, tc, x_d.ap(), mp_d.ap(), r_d.ap(), id_d.ap(),
                  io_d.ap(), out_d.ap())
    nc.compile()
    return nc


def get_nc():
    if "nc" not in _cache:
        _cache["nc"] = _build_program()
    return _cache["nc"]


def make_in_maps(x, mask_positions, W, b, mask_token):
    xf = np.ascontiguousarray(np.asarray(x, dtype=np.float32))
    mpf = np.ascontiguousarray(np.asarray(mask_positions).astype(np.float32))
    Wf = np.asarray(W, dtype=np.float32)
    bf = np.asarray(b, dtype=np.float32)
    mtf = np.asarray(mask_token, dtype=np.float32)

    R_host = np.zeros((12, NI * D), dtype=np.float32)
    for q in range(4):
        for i in range(NI):
            f = 4 * i + q
            R_host[3 * q + 0, i * D:(i + 1) * D] = Wf[f]
            R_host[3 * q + 1, i * D:(i + 1) * D] = bf[f]
            R_host[3 * q + 2, i * D:(i + 1) * D] = mtf
    ident = np.eye(128, dtype=np.float32)
    iota = np.broadcast_to(np.arange(F, dtype=np.float32), (128, F)).copy()

    maps = []
    for cc in range(NCORES):
        sl = slice(cc * B_LOC, (cc + 1) * B_LOC)
        maps.append({"x_in": xf[sl], "mp_in": mpf[sl], "r_in": R_host,
                     "id_in": ident, "io_in": iota})
    return maps


def kernel(x, mask_positions, W, b, mask_token):
    nc = get_nc()
    maps = make_in_maps(x, mask_positions, W, b, mask_token)
    res = run_bass_kernel_spmd(nc, maps, core_ids=list(range(NCORES)))
    return np.concatenate([res.results[cc]["out"] for cc in range(NCORES)],
                          axis=0)


# revision 34
# speedup vs baseline: 1.0403x; 1.0403x over previous
"""Trainium2 Bass kernel for FeatureEmbedding (embedding_lookup).

Computes, for inputs x[B,F], mask_positions[B,NM], W[F,D], b[F,D], mask_token[D]:

    tokens[bb, f, :] = x[bb, f] * W[f, :] + b[f, :]
    tokens[bb, p, :] = mask_token          for p in mask_positions[bb]

Strategy (data-parallel over 8 NeuronCores, 256 batch rows each):
  - On device, build a one-hot mask m[bb,f] = any_j(mask_positions[bb,j]==f)
    with iota/is_equal vector ops, then om = 1-m and xs = x*om.
  - For each feature f the output tile out[:, f, :] (128 batch rows x 512) is
    a rank-3 product computed on the TensorEngine:
        out = xs[:,f] (x) W[f,:] + om[:,f] (x) b[f,:] + m[:,f] (x) mask_token
    i.e. a K=3 matmul with stationary rows (xs_f, om_f, m_f) and moving rows
    (W[f], b[f], mask_token).  The stationary triples are produced by PE
    transposes of a packed tile; four features run concurrently in the four
    32-row PE tile groups (row tiling).  PSUM tiles are evicted to SBUF by
    the Vector and Scalar engines (split), then streamed to HBM in 2 MiB DMAs.
  - Matmul operands use float32r (same 4-byte layout as f32; fast PE path).

The output write (32 MiB/core, 256 MiB total) is the roofline term.
"""

import dataclasses
import os
from contextlib import ExitStack

import numpy as np

import concourse.bass as bass
import concourse.tile as tile
from concourse import bacc, mybir
from concourse.bass_utils import run_bass_kernel_spmd

B, F, D, NM = 2048, 64, 512, 8
NCORES = 8
B_LOC = B // NCORES          # 256 batch rows per core
NT = B_LOC // 128            # 2 partition-tiles of 128 batch rows
NI = F // 4                  # 16 stationary packs (4 features each)

F32 = mybir.dt.float32
# Output chunking: feature spans per output DMA (small first for fast start).
CHUNKS = [(0, 2), (2, 2), (4, 4)] + [(8 * k, 8) for k in range(1, 8)]
# float32r: same bits as f32, streams through the PE at 1 cycle/row
# (fp32 proper needs 4).  Flip to False if precision ever becomes an issue.
USE_F32R = os.environ.get("KERNEL_NO_F32R", "") == ""
FMM = mybir.dt.float32r if USE_F32R else mybir.dt.float32
AOT = mybir.AluOpType

_cache: dict = {}


def _body(ctx: ExitStack, tc: tile.TileContext, x_ap, mp_ap, r_ap, id_ap,
          io_ap, out_ap):
    nc = tc.nc
    singles = ctx.enter_context(tc.tile_pool(name="singles", bufs=1))
    eqpool = ctx.enter_context(tc.tile_pool(name="eqp", bufs=2))
    spool = ctx.enter_context(tc.tile_pool(name="spool", bufs=20))
    stpool = ctx.enter_context(tc.tile_pool(name="stage", bufs=6))
    tpsum = ctx.enter_context(tc.tile_pool(name="tpsum", bufs=4, space="PSUM"))
    opsum = ctx.enter_context(tc.tile_pool(name="opsum", bufs=4, space="PSUM"))

    # All input DMAs issue up-front on the HWDGE (sync) path: the batch-tile
    # inputs first (they gate the first output chunk), then params/constants.
    # Input DMAs on the sync HWDGE ring, ordered by when the pipeline needs
    # them: the mask chain (mp, iota, x) gates everything else.
    xts = [singles.tile([128, F], F32, tag=f"xt{t}", name=f"xt{t}")
           for t in range(NT)]
    mpts = [singles.tile([128, NM], F32, tag=f"mpt{t}", name=f"mpt{t}")
            for t in range(NT)]
    iota = singles.tile([128, F], F32, tag="iota")
    ident = singles.tile([128, 128], F32, tag="ident")
    nc.sync.dma_start(out=mpts[0][:], in_=mp_ap[0:128, :])
    nc.sync.dma_start(out=iota[:], in_=io_ap[:, :])
    nc.sync.dma_start(out=xts[0][:], in_=x_ap[0:128, :])
    nc.sync.dma_start(out=ident[:], in_=id_ap[:, :])
    for t in range(1, NT):
        nc.sync.dma_start(out=mpts[t][:], in_=mp_ap[t * 128:(t + 1) * 128, :])
        nc.sync.dma_start(out=xts[t][:], in_=x_ap[t * 128:(t + 1) * 128, :])

    # R holds the moving operands: PE row-group q serves features f = q mod 4;
    # partitions 32q+{0,1,2} hold (W[f], b[f], mask_token) of f = 4i+q at
    # free cols i*D.  The 3-partition transfers are slow, so they ride the
    # scalar-engine HWDGE ring (separate FIFO from the output ring) as a
    # column ladder: the first four small DMAs unlock output chunk 0, the
    # tails stream in behind the early chunks.
    R = singles.tile([128, NI * D], FMM, tag="R")
    for q in range(4):
        nc.scalar.dma_start(out=R[32 * q:32 * q + 3, 0:4 * D],
                            in_=r_ap[3 * q:3 * q + 3, 0:4 * D])
    for q in range(4):
        nc.scalar.dma_start(out=R[32 * q:32 * q + 3, 4 * D:],
                            in_=r_ap[3 * q:3 * q + 3, 4 * D:])

    # P (one per batch-tile, ping-pong) packs the per-batch stationary data in
    # natural layout so one PE transpose per 128 columns yields the stationary
    # tile: column 128i + 32g + c = component c (xs, om, m) of feature 4i+g.
    # Columns 32g+3..32g+31 are never written: they land in stationary rows
    # the K=3 matmuls never read, so no zeroing is needed.
    Ps = [singles.tile([128, NI * 128], F32, tag=f"P{t}", name=f"P{t}")
          for t in range(NT)]

    # Feature map: f = 4i + g -> stationary pack S_i, PE row-group g.
    iov = iota[:, :].rearrange("p (i g) -> p i g", g=4)

    for T in range(NT):
        xt = xts[T]
        mpt = mpts[T]
        P = Ps[T]
        Pv = P[:, :].rearrange("p (i g c) -> p i g c", g=4, c=32)
        xsv = Pv[:, :, :, 0]
        omv = Pv[:, :, :, 1]
        mv = Pv[:, :, :, 2]
        xv = xt[:, :].rearrange("p (i g) -> p i g", g=4)

        # m = one-hot of mask positions: compare iota against every index
        # column in one broadcast op, then max-reduce over the index axis.
        eq = eqpool.tile([128, F * NM], F32, tag="eq")
        eqv = eq[:, :].rearrange("p (i g j) -> p i g j", g=4, j=NM)
        iov4 = iov.to_broadcast([128, NI, 4, NM])
        mpv4 = dataclasses.replace(
            mpt[:, :], ap=[[NM, 128], [0, NI], [0, 4], [1, NM]])
        nc.vector.tensor_tensor(eqv, iov4, mpv4, op=AOT.is_equal)
        nc.vector.tensor_reduce(mv, eqv, axis=mybir.AxisListType.X,
                                op=AOT.max)
        nc.vector.tensor_scalar(omv, mv, -1.0, 1.0, AOT.mult, AOT.add)
        nc.vector.tensor_tensor(xsv, xv, omv, op=AOT.mult)

        # Transpose the 16 packs into stationary tiles.
        Ss = []
        for i in range(NI):
            pst = tpsum.tile([128, 128], F32, tag="pst")
            nc.tensor.transpose(pst[:], P[:, 128 * i:128 * (i + 1)], ident[:])
            S = spool.tile([128, 128], FMM, tag="S")
            if i % 2 == 0:
                nc.vector.tensor_copy(S[:], pst[:])
            else:
                nc.scalar.copy(S[:], pst[:])
            Ss.append(S)

        # 64 K=3 row-tiled matmuls; evict PSUM->SBUF stage; DMA out per
        # chunk.  The first chunks are small so the output stream starts as
        # early as possible.
        for ci, (s0, nf) in enumerate(CHUNKS):
            stg = stpool.tile([128, 8 * D], F32, tag="stg")
            for k in range(nf):
                f = s0 + k
                i, g = divmod(f, 4)
                po = opsum.tile([128, D], F32, tag="po")
                nc.tensor.matmul(po[:], lhsT=Ss[i][32 * g:32 * g + 3, :],
                                 rhs=R[32 * g:32 * g + 3, bass.ts(i, D)],
                                 start=True, stop=True,
                                 tile_position=(32 * g, 0))
                if f % 2 == 0:
                    nc.vector.tensor_copy(stg[:, bass.ts(k, D)], po[:])
                else:
                    nc.scalar.copy(stg[:, bass.ts(k, D)], po[:])
            nc.sync.dma_start(
                out=out_ap[T * 128:(T + 1) * 128, s0:s0 + nf, :],
                in_=stg[:, 0:nf * D].rearrange("p (a b) -> p a b", b=D))


def _build_program():
    nc = bacc.Bacc("TRN2", target_bir_lowering=False, debug=False,
                   enable_asserts=True, num_devices=NCORES)
    x_d = nc.dram_tensor("x_in", [B_LOC, F], F32, kind="ExternalInput")
    mp_d = nc.dram_tensor("mp_in", [B_LOC, NM], F32, kind="ExternalInput")
    r_d = nc.dram_tensor("r_in", [12, NI * D], FMM, kind="ExternalInput")
    id_d = nc.dram_tensor("id_in", [128, 128], F32, kind="ExternalInput")
    io_d = nc.dram_tensor("io_in", [128, F], F32, kind="ExternalInput")
    out_d = nc.dram_tensor("out", [B_LOC, F, D], F32, kind="ExternalOutput")

    with tile.TileContext(nc) as tc:
        with ExitStack() as ctx:
            _body(ctx, tc, x_d.ap(), mp_d.ap(), r_d.ap(), id_d.ap(),
                  io_d.ap(), out_d.ap())
    nc.compile()
    return nc


def get_nc():
    if "nc" not in _cache:
        _cache["nc"] = _build_program()
    return _cache["nc"]


def make_in_maps(x, mask_positions, W, b, mask_token):
    xf = np.ascontiguousarray(np.asarray(x, dtype=np.float32))
    mpf = np.ascontiguousarray(np.asarray(mask_positions).astype(np.float32))
    Wf = np.asarray(W, dtype=np.float32)
    bf = np.asarray(b, dtype=np.float32)
    mtf = np.asarray(mask_token, dtype=np.float32)

    R_host = np.zeros((12, NI * D), dtype=np.float32)
    for q in range(4):
        for i in range(NI):
            f = 4 * i + q
            R_host[3 * q + 0, i * D:(i + 1) * D] = Wf[f]
            R_host[3 * q + 1, i * D:(i + 1) * D] = bf[f]
            R_host[3 * q + 2, i * D:(i + 1) * D] = mtf
    ident = np.eye(128, dtype=np.float32)
    iota = np.broadcast_to(np.arange(F, dtype=np.float32), (128, F)).copy()

    maps = []
    for cc in range(NCORES):
        sl = slice(cc * B_LOC, (cc + 1) * B_LOC)
        maps.append({"x_in": xf[sl], "mp_in": mpf[sl], "r_in": R_host,
                     "id_in": ident, "io_in": iota})
    return maps


def kernel(x, mask_positions, W, b, mask_token):
    nc = get_nc()
    maps = make_in_maps(x, mask_positions, W, b, mask_token)
    res = run_bass_kernel_spmd(nc, maps, core_ids=list(range(NCORES)))
    return np.concatenate([res.results[cc]["out"] for cc in range(NCORES)],
                          axis=0)


# revision 36
# speedup vs baseline: 1.2031x; 1.1564x over previous
"""Trainium2 Bass kernel for FeatureEmbedding (embedding_lookup).

Computes, for inputs x[B,F], mask_positions[B,NM], W[F,D], b[F,D], mask_token[D]:

    tokens[bb, f, :] = x[bb, f] * W[f, :] + b[f, :]
    tokens[bb, p, :] = mask_token          for p in mask_positions[bb]

Strategy (data-parallel over 8 NeuronCores, 256 batch rows each):
  - On device, build a one-hot mask m[bb,f] = any_j(mask_positions[bb,j]==f)
    with iota/is_equal vector ops, then om = 1-m and xs = x*om.
  - For each feature f the output tile out[:, f, :] (128 batch rows x 512) is
    a rank-3 product computed on the TensorEngine:
        out = xs[:,f] (x) W[f,:] + om[:,f] (x) b[f,:] + m[:,f] (x) mask_token
    i.e. a K=3 matmul with stationary rows (xs_f, om_f, m_f) and moving rows
    (W[f], b[f], mask_token).  The stationary triples are produced by PE
    transposes of a packed tile; four features run concurrently in the four
    32-row PE tile groups (row tiling).  PSUM tiles are evicted to SBUF by
    the Vector and Scalar engines (split), then streamed to HBM in 2 MiB DMAs.
  - Matmul operands use float32r (same 4-byte layout as f32; fast PE path).

The output write (32 MiB/core, 256 MiB total) is the roofline term.
"""

import dataclasses
import os
from contextlib import ExitStack

import numpy as np

import concourse.bass as bass
import concourse.tile as tile
from concourse import bacc, mybir
from concourse.bass_utils import run_bass_kernel_spmd

B, F, D, NM = 2048, 64, 512, 8
NCORES = 8
B_LOC = B // NCORES          # 256 batch rows per core
NT = B_LOC // 128            # 2 partition-tiles of 128 batch rows
NI = F // 4                  # 16 stationary packs (4 features each)

F32 = mybir.dt.float32
# Output chunking: feature spans per output DMA.
CHUNKS = [(8 * k, 8) for k in range(8)]
# float32r: same bits as f32, streams through the PE at 1 cycle/row
# (fp32 proper needs 4).  Flip to False if precision ever becomes an issue.
USE_F32R = os.environ.get("KERNEL_NO_F32R", "") == ""
FMM = mybir.dt.float32r if USE_F32R else mybir.dt.float32
AOT = mybir.AluOpType

_cache: dict = {}


def _body(ctx: ExitStack, tc: tile.TileContext, x_ap, mp_ap, r_ap, id_ap,
          io_ap, out_ap):
    nc = tc.nc
    singles = ctx.enter_context(tc.tile_pool(name="singles", bufs=1))
    eqpool = ctx.enter_context(tc.tile_pool(name="eqp", bufs=2))
    spool = ctx.enter_context(tc.tile_pool(name="spool", bufs=20))
    stpool = ctx.enter_context(tc.tile_pool(name="stage", bufs=4))
    tpsum = ctx.enter_context(tc.tile_pool(name="tpsum", bufs=4, space="PSUM"))
    opsum = ctx.enter_context(tc.tile_pool(name="opsum", bufs=4, space="PSUM"))

    # All input DMAs issue up-front on the HWDGE (sync) path: the batch-tile
    # inputs first (they gate the first output chunk), then params/constants.
    # Input DMAs on the sync HWDGE ring, ordered by when the pipeline needs
    # them: the mask chain (mp, iota, x) gates everything else.
    xts = [singles.tile([128, F], F32, tag=f"xt{t}", name=f"xt{t}")
           for t in range(NT)]
    mpts = [singles.tile([128, NM], F32, tag=f"mpt{t}", name=f"mpt{t}")
            for t in range(NT)]
    iota = singles.tile([128, F], F32, tag="iota")
    ident = singles.tile([128, 128], F32, tag="ident")
    nc.sync.dma_start(out=mpts[0][:], in_=mp_ap[0:128, :])
    nc.sync.dma_start(out=iota[:], in_=io_ap[:, :])
    nc.sync.dma_start(out=xts[0][:], in_=x_ap[0:128, :])
    nc.sync.dma_start(out=ident[:], in_=id_ap[:, :])
    for t in range(1, NT):
        nc.sync.dma_start(out=mpts[t][:], in_=mp_ap[t * 128:(t + 1) * 128, :])
        nc.sync.dma_start(out=xts[t][:], in_=x_ap[t * 128:(t + 1) * 128, :])

    # R holds the moving operands: PE row-group q serves features f = q mod 4;
    # partitions 32q+{0,1,2} hold (W[f], b[f], mask_token) of f = 4i+q at
    # free cols i*D.  The 3-partition transfers are slow, so they ride the
    # scalar-engine HWDGE ring (separate FIFO from the output ring) as a
    # column ladder: the first four small DMAs unlock output chunk 0, the
    # tails stream in behind the early chunks.
    R = singles.tile([128, NI * D], FMM, tag="R")
    for q in range(4):
        nc.scalar.dma_start(out=R[32 * q:32 * q + 3, 0:4 * D],
                            in_=r_ap[3 * q:3 * q + 3, 0:4 * D])
    for q in range(4):
        nc.scalar.dma_start(out=R[32 * q:32 * q + 3, 4 * D:],
                            in_=r_ap[3 * q:3 * q + 3, 4 * D:])

    # P (one per batch-tile, ping-pong) packs the per-batch stationary data in
    # natural layout so one PE transpose per 128 columns yields the stationary
    # tile: column 128i + 32g + c = component c (xs, om, m) of feature 4i+g.
    # Columns 32g+3..32g+31 are never written: they land in stationary rows
    # the K=3 matmuls never read, so no zeroing is needed.
    Ps = [singles.tile([128, NI * 128], F32, tag=f"P{t}", name=f"P{t}")
          for t in range(NT)]

    # Feature map: f = 4i + g -> stationary pack S_i, PE row-group g.
    iov = iota[:, :].rearrange("p (i g) -> p i g", g=4)

    for T in range(NT):
        xt = xts[T]
        mpt = mpts[T]
        P = Ps[T]
        Pv = P[:, :].rearrange("p (i g c) -> p i g c", g=4, c=32)
        xsv = Pv[:, :, :, 0]
        omv = Pv[:, :, :, 1]
        mv = Pv[:, :, :, 2]
        xv = xt[:, :].rearrange("p (i g) -> p i g", g=4)

        # m = one-hot of mask positions: compare iota against every index
        # column in one broadcast op, then max-reduce over the index axis.
        eq = eqpool.tile([128, F * NM], F32, tag="eq")
        eqv = eq[:, :].rearrange("p (i g j) -> p i g j", g=4, j=NM)
        iov4 = iov.to_broadcast([128, NI, 4, NM])
        mpv4 = dataclasses.replace(
            mpt[:, :], ap=[[NM, 128], [0, NI], [0, 4], [1, NM]])
        nc.vector.tensor_tensor(eqv, iov4, mpv4, op=AOT.is_equal)
        nc.vector.tensor_reduce(mv, eqv, axis=mybir.AxisListType.X,
                                op=AOT.max)
        nc.vector.tensor_scalar(omv, mv, -1.0, 1.0, AOT.mult, AOT.add)
        nc.vector.tensor_tensor(xsv, xv, omv, op=AOT.mult)

        # Transpose the 16 packs into stationary tiles.
        Ss = []
        for i in range(NI):
            pst = tpsum.tile([128, 128], F32, tag="pst")
            nc.tensor.transpose(pst[:], P[:, 128 * i:128 * (i + 1)], ident[:])
            S = spool.tile([128, 128], FMM, tag="S")
            if i % 2 == 0:
                nc.vector.tensor_copy(S[:], pst[:])
            else:
                nc.scalar.copy(S[:], pst[:])
            Ss.append(S)

        # 64 K=3 row-tiled matmuls; evict PSUM->SBUF stage; DMA out per
        # chunk.  The first chunks are small so the output stream starts as
        # early as possible.
        for ci, (s0, nf) in enumerate(CHUNKS):
            stg = stpool.tile([128, 8 * D], F32, tag="stg")
            for k in range(nf):
                f = s0 + k
                i, g = divmod(f, 4)
                po = opsum.tile([128, D], F32, tag="po")
                nc.tensor.matmul(po[:], lhsT=Ss[i][32 * g:32 * g + 3, :],
                                 rhs=R[32 * g:32 * g + 3, bass.ts(i, D)],
                                 start=True, stop=True,
                                 tile_position=(32 * g, 0))
                if f % 2 == 0:
                    nc.vector.tensor_copy(stg[:, bass.ts(k, D)], po[:])
                else:
                    nc.scalar.copy(stg[:, bass.ts(k, D)], po[:])
            nc.sync.dma_start(
                out=out_ap[T * 128:(T + 1) * 128, s0:s0 + nf, :],
                in_=stg[:, 0:nf * D].rearrange("p (a b) -> p a b", b=D))


def _build_program():
    nc = bacc.Bacc("TRN2", target_bir_lowering=False, debug=False,
                   enable_asserts=True, num_devices=NCORES)
    x_d = nc.dram_tensor("x_in", [B_LOC, F], F32, kind="ExternalInput")
    mp_d = nc.dram_tensor("mp_in", [B_LOC, NM], F32, kind="ExternalInput")
    r_d = nc.dram_tensor("r_in", [12, NI * D], FMM, kind="ExternalInput")
    id_d = nc.dram_tensor("id_in", [128, 128], F32, kind="ExternalInput")
    io_d = nc.dram_tensor("io_in", [128, F], F32, kind="ExternalInput")
    out_d = nc.dram_tensor("out", [B_LOC, F, D], F32, kind="ExternalOutput")

    with tile.TileContext(nc) as tc:
        with ExitStack() as ctx:
            _body(ctx, tc, x_d.ap(), mp_d.ap(), r_d.ap(), id_d.ap(),
                  io_d.ap(), out_d.ap())
    nc.compile()
    return nc


def get_nc():
    if "nc" not in _cache:
        _cache["nc"] = _build_program()
    return _cache["nc"]


def make_in_maps(x, mask_positions, W, b, mask_token):
    xf = np.ascontiguousarray(np.asarray(x, dtype=np.float32))
    mpf = np.ascontiguousarray(np.asarray(mask_positions).astype(np.float32))
    Wf = np.asarray(W, dtype=np.float32)
    bf = np.asarray(b, dtype=np.float32)
    mtf = np.asarray(mask_token, dtype=np.float32)

    R_host = np.zeros((12, NI * D), dtype=np.float32)
    for q in range(4):
        for i in range(NI):
            f = 4 * i + q
            R_host[3 * q + 0, i * D:(i + 1) * D] = Wf[f]
            R_host[3 * q + 1, i * D:(i + 1) * D] = bf[f]
            R_host[3 * q + 2, i * D:(i + 1) * D] = mtf
    ident = np.eye(128, dtype=np.float32)
    iota = np.broadcast_to(np.arange(F, dtype=np.float32), (128, F)).copy()

    maps = []
    for cc in range(NCORES):
        sl = slice(cc * B_LOC, (cc + 1) * B_LOC)
        maps.append({"x_in": xf[sl], "mp_in": mpf[sl], "r_in": R_host,
                     "id_in": ident, "io_in": iota})
    return maps


def kernel(x, mask_positions, W, b, mask_token):
    nc = get_nc()
    maps = make_in_maps(x, mask_positions, W, b, mask_token)
    res = run_bass_kernel_spmd(nc, maps, core_ids=list(range(NCORES)))
    return np.concatenate([res.results[cc]["out"] for cc in range(NCORES)],
                          axis=0)


# revision 51
# speedup vs baseline: 1.2053x; 1.0018x over previous
"""Trainium2 Bass kernel for FeatureEmbedding (embedding_lookup).

Computes, for inputs x[B,F], mask_positions[B,NM], W[F,D], b[F,D], mask_token[D]:

    tokens[bb, f, :] = x[bb, f] * W[f, :] + b[f, :]
    tokens[bb, p, :] = mask_token          for p in mask_positions[bb]

Strategy (data-parallel over 8 NeuronCores, 256 batch rows each):
  - On device, build a one-hot mask m[bb,f] = any_j(mask_positions[bb,j]==f)
    with iota/is_equal vector ops, then om = 1-m and xs = x*om.
  - For each feature f the output tile out[:, f, :] (128 batch rows x 512) is
    a rank-3 product computed on the TensorEngine:
        out = xs[:,f] (x) W[f,:] + om[:,f] (x) b[f,:] + m[:,f] (x) mask_token
    i.e. a K=3 matmul with stationary rows (xs_f, om_f, m_f) and moving rows
    (W[f], b[f], mask_token).  The stationary triples are produced by PE
    transposes of a packed tile; four features run concurrently in the four
    32-row PE tile groups (row tiling).  PSUM tiles are evicted to SBUF by
    the Vector and Scalar engines (split), then streamed to HBM in 2 MiB DMAs.
  - Matmul operands use float32r (same 4-byte layout as f32; fast PE path).

The output write (32 MiB/core, 256 MiB total) is the roofline term.
"""

import dataclasses
import os
from contextlib import ExitStack

import numpy as np

import concourse.bass as bass
import concourse.tile as tile
from concourse import bacc, mybir
from concourse.bass_utils import run_bass_kernel_spmd

B, F, D, NM = 2048, 64, 512, 8
NCORES = 8
B_LOC = B // NCORES          # 256 batch rows per core
NT = B_LOC // 128            # 2 partition-tiles of 128 batch rows
NI = F // 4                  # 16 stationary packs (4 features each)

F32 = mybir.dt.float32
# Output chunking: feature spans per output DMA.
CHUNKS = [(8 * k, 8) for k in range(8)]
# Packed small-input width: iota | (mp, x) per batch-tile | identity.
MISC_W = F + NT * (NM + F) + 128
# float32r: same bits as f32, streams through the PE at 1 cycle/row
# (fp32 proper needs 4).  Flip to False if precision ever becomes an issue.
USE_F32R = os.environ.get("KERNEL_NO_F32R", "") == ""
FMM = mybir.dt.float32r if USE_F32R else mybir.dt.float32
AOT = mybir.AluOpType

_cache: dict = {}


def _body(ctx: ExitStack, tc: tile.TileContext, x_ap, r_ap, out_ap):
    nc = tc.nc
    singles = ctx.enter_context(tc.tile_pool(name="singles", bufs=1))
    eqpool = ctx.enter_context(tc.tile_pool(name="eqp", bufs=2))
    spool = ctx.enter_context(tc.tile_pool(name="spool", bufs=20))
    stpool = ctx.enter_context(tc.tile_pool(name="stage", bufs=4))
    tpsum = ctx.enter_context(tc.tile_pool(name="tpsum", bufs=4, space="PSUM"))
    opsum = ctx.enter_context(tc.tile_pool(name="opsum", bufs=4, space="PSUM"))

    # All the small inputs (iota, mask positions, x, identity) arrive
    # host-packed in one tensor so a single sync-ring DMA (one completion
    # wait) delivers everything the mask chain needs.
    misc = singles.tile([128, MISC_W], F32, tag="misc")
    nc.sync.dma_start(out=misc[:], in_=x_ap[:, :])
    iota = misc[:, 0:F]
    o = F
    mpts, xts = [], []
    for t in range(NT):
        mpts.append(misc[:, o:o + NM])
        xts.append(misc[:, o + NM:o + NM + F])
        o += NM + F
    ident = misc[:, o:o + 128]

    # R holds the moving operands: PE row-group q serves features f = q mod 4;
    # partitions 32q+{0,1,2} hold (W[f], b[f], mask_token) of f = 4i+q at
    # free cols i*D.  The 3-partition transfers are slow, so they ride the
    # scalar-engine HWDGE ring (a separate FIFO - on the sync ring they would
    # stall the output chunks queued behind them) in two column waves: the
    # first wave unlocks the early output chunks, the tail streams in behind.
    R = singles.tile([128, NI * D], FMM, tag="R")
    for q in range(4):
        nc.scalar.dma_start(out=R[32 * q:32 * q + 3, 0:4 * D],
                            in_=r_ap[3 * q:3 * q + 3, 0:4 * D])
    for q in range(4):
        nc.scalar.dma_start(out=R[32 * q:32 * q + 3, 4 * D:],
                            in_=r_ap[3 * q:3 * q + 3, 4 * D:])

    # P (one per batch-tile, ping-pong) packs the per-batch stationary data in
    # natural layout so one PE transpose per 128 columns yields the stationary
    # tile: column 128i + 32g + c = component c (xs, om, m) of feature 4i+g.
    # Columns 32g+3..32g+31 are never written: they land in stationary rows
    # the K=3 matmuls never read, so no zeroing is needed.
    Ps = [singles.tile([128, NI * 128], F32, tag=f"P{t}", name=f"P{t}")
          for t in range(NT)]

    # Feature map: f = 4i + g -> stationary pack S_i, PE row-group g.
    iov = iota[:, :].rearrange("p (i g) -> p i g", g=4)

    for T in range(NT):
        xt = xts[T]
        mpt = mpts[T]
        P = Ps[T]
        Pv = P[:, :].rearrange("p (i g c) -> p i g c", g=4, c=32)
        xsv = Pv[:, :, :, 0]
        omv = Pv[:, :, :, 1]
        mv = Pv[:, :, :, 2]
        xv = xt.rearrange("p (i g) -> p i g", g=4)

        # m = one-hot of mask positions: compare iota against every index
        # column in one broadcast op, then max-reduce over the index axis.
        eq = eqpool.tile([128, F * NM], F32, tag="eq")
        eqv = eq[:, :].rearrange("p (i g j) -> p i g j", g=4, j=NM)
        iov4 = iov.to_broadcast([128, NI, 4, NM])
        mpv4 = dataclasses.replace(
            mpt, ap=[[mpt.ap[0][0], 128], [0, NI], [0, 4], [1, NM]])
        nc.vector.tensor_tensor(eqv, iov4, mpv4, op=AOT.is_equal)
        nc.vector.tensor_reduce(mv, eqv, axis=mybir.AxisListType.X,
                                op=AOT.max)
        nc.vector.tensor_scalar(omv, mv, -1.0, 1.0, AOT.mult, AOT.add)
        nc.vector.tensor_tensor(xsv, xv, omv, op=AOT.mult)

        # PE transpose pack i -> stationary tile S_i (evicted by DVE/ACT).
        Ss = [None] * NI

        def make_S(i):
            pst = tpsum.tile([128, 128], F32, tag="pst", name="pst")
            nc.tensor.transpose(pst[:], P[:, 128 * i:128 * (i + 1)], ident)
            S = spool.tile([128, 128], FMM, tag="S", name="S")
            if i % 2 == 0:
                nc.vector.tensor_copy(S[:], pst[:])
            else:
                nc.scalar.copy(S[:], pst[:])
            Ss[i] = S

        # 8 K=3 row-tiled matmuls per chunk; evict PSUM->SBUF stage; DMA out.
        def do_chunk(ci):
            s0, nf = CHUNKS[ci]
            stg = stpool.tile([128, 8 * D], F32, tag="stg", name="stg")
            for k in range(nf):
                f = s0 + k
                i, g = divmod(f, 4)
                po = opsum.tile([128, D], F32, tag="po", name="po")
                nc.tensor.matmul(po[:], lhsT=Ss[i][32 * g:32 * g + 3, :],
                                 rhs=R[32 * g:32 * g + 3, bass.ts(i, D)],
                                 start=True, stop=True,
                                 tile_position=(32 * g, 0))
                if f % 2 == 0:
                    nc.vector.tensor_copy(stg[:, bass.ts(k, D)], po[:])
                else:
                    nc.scalar.copy(stg[:, bass.ts(k, D)], po[:])
            nc.sync.dma_start(
                out=out_ap[T * 128:(T + 1) * 128, s0:s0 + nf, :],
                in_=stg[:, 0:nf * D].rearrange("p (a b) -> p a b", b=D))

        for i in range(NI):
            make_S(i)
        for ci in range(len(CHUNKS)):
            do_chunk(ci)


def _build_program():
    nc = bacc.Bacc("TRN2", target_bir_lowering=False, debug=False,
                   enable_asserts=True, num_devices=NCORES)
    misc_d = nc.dram_tensor("misc_in", [128, MISC_W], F32,
                            kind="ExternalInput")
    r_d = nc.dram_tensor("r_in", [12, NI * D], FMM, kind="ExternalInput")
    out_d = nc.dram_tensor("out", [B_LOC, F, D], F32, kind="ExternalOutput")

    with tile.TileContext(nc) as tc:
        with ExitStack() as ctx:
            _body(ctx, tc, misc_d.ap(), r_d.ap(), out_d.ap())
    nc.compile()
    return nc


def get_nc():
    if "nc" not in _cache:
        _cache["nc"] = _build_program()
    return _cache["nc"]


def make_in_maps(x, mask_positions, W, b, mask_token):
    xf = np.ascontiguousarray(np.asarray(x, dtype=np.float32))
    mpf = np.ascontiguousarray(np.asarray(mask_positions).astype(np.float32))
    Wf = np.asarray(W, dtype=np.float32)
    bf = np.asarray(b, dtype=np.float32)
    mtf = np.asarray(mask_token, dtype=np.float32)

    R_host = np.zeros((12, NI * D), dtype=np.float32)
    for q in range(4):
        for i in range(NI):
            f = 4 * i + q
            R_host[3 * q + 0, i * D:(i + 1) * D] = Wf[f]
            R_host[3 * q + 1, i * D:(i + 1) * D] = bf[f]
            R_host[3 * q + 2, i * D:(i + 1) * D] = mtf
    ident = np.eye(128, dtype=np.float32)
    iota = np.broadcast_to(np.arange(F, dtype=np.float32), (128, F))

    maps = []
    for cc in range(NCORES):
        s = cc * B_LOC
        parts = [iota]
        for t in range(NT):
            parts.append(mpf[s + t * 128:s + (t + 1) * 128])
            parts.append(xf[s + t * 128:s + (t + 1) * 128])
        parts.append(ident)
        misc = np.ascontiguousarray(
            np.concatenate(parts, axis=1, dtype=np.float32))
        assert misc.shape == (128, MISC_W)
        maps.append({"misc_in": misc, "r_in": R_host})
    return maps


def kernel(x, mask_positions, W, b, mask_token):
    nc = get_nc()
    maps = make_in_maps(x, mask_positions, W, b, mask_token)
    res = run_bass_kernel_spmd(nc, maps, core_ids=list(range(NCORES)))
    return np.concatenate([res.results[cc]["out"] for cc in range(NCORES)],
                          axis=0)


# revision 59
# speedup vs baseline: 1.2202x; 1.0124x over previous
"""Trainium2 Bass kernel for FeatureEmbedding (embedding_lookup).

Computes, for inputs x[B,F], mask_positions[B,NM], W[F,D], b[F,D], mask_token[D]:

    tokens[bb, f, :] = x[bb, f] * W[f, :] + b[f, :]
    tokens[bb, p, :] = mask_token          for p in mask_positions[bb]

Strategy (data-parallel over 8 NeuronCores, 256 batch rows each):
  - On device, build a one-hot mask m[bb,f] = any_j(mask_positions[bb,j]==f)
    with iota/is_equal vector ops, then om = 1-m and xs = x*om.
  - For each feature f the output tile out[:, f, :] (128 batch rows x 512) is
    a rank-3 product computed on the TensorEngine:
        out = xs[:,f] (x) W[f,:] + om[:,f] (x) b[f,:] + m[:,f] (x) mask_token
    i.e. a K=3 matmul with stationary rows (xs_f, om_f, m_f) and moving rows
    (W[f], b[f], mask_token).  The stationary triples are produced by PE
    transposes of a packed tile; four features run concurrently in the four
    32-row PE tile groups (row tiling).  PSUM tiles are evicted to SBUF by
    the Vector and Scalar engines (split), then streamed to HBM in 2 MiB DMAs.
  - Matmul operands use float32r (same 4-byte layout as f32; fast PE path).

The output write (32 MiB/core, 256 MiB total) is the roofline term.
"""

import dataclasses
import os
from contextlib import ExitStack

import numpy as np

import concourse.bass as bass
import concourse.tile as tile
from concourse import bacc, mybir
from concourse.bass_utils import run_bass_kernel_spmd

B, F, D, NM = 2048, 64, 512, 8
NCORES = 8
B_LOC = B // NCORES          # 256 batch rows per core
NT = B_LOC // 128            # 2 partition-tiles of 128 batch rows
NI = F // 4                  # 16 stationary packs (4 features each)

F32 = mybir.dt.float32
# Output chunking: feature spans per output DMA.
CHUNKS = [(8 * k, 8) for k in range(8)]
# Packed small-input width: iota | (mp, x) per batch-tile | identity.
MISC_W = F + NT * (NM + F) + 128
# float32r: same bits as f32, streams through the PE at 1 cycle/row
# (fp32 proper needs 4).  Flip to False if precision ever becomes an issue.
USE_F32R = os.environ.get("KERNEL_NO_F32R", "") == ""
FMM = mybir.dt.float32r if USE_F32R else mybir.dt.float32
AOT = mybir.AluOpType

_cache: dict = {}


def _body(ctx: ExitStack, tc: tile.TileContext, x_ap, r_ap, out_ap):
    nc = tc.nc
    singles = ctx.enter_context(tc.tile_pool(name="singles", bufs=1))
    eqpool = ctx.enter_context(tc.tile_pool(name="eqp", bufs=2))
    spool = ctx.enter_context(tc.tile_pool(name="spool", bufs=20))
    stpool = ctx.enter_context(tc.tile_pool(name="stage", bufs=4))
    tpsum = ctx.enter_context(tc.tile_pool(name="tpsum", bufs=2, space="PSUM"))
    opsum = ctx.enter_context(tc.tile_pool(name="opsum", bufs=3, space="PSUM"))

    # All the small inputs (iota, mask positions, x, identity) arrive
    # host-packed in one tensor so a single sync-ring DMA (one completion
    # wait) delivers everything the mask chain needs.
    misc = singles.tile([128, MISC_W], F32, tag="misc")
    nc.sync.dma_start(out=misc[:], in_=x_ap[:, :])
    iota = misc[:, 0:F]
    o = F
    mpts, xts = [], []
    for t in range(NT):
        mpts.append(misc[:, o:o + NM])
        xts.append(misc[:, o + NM:o + NM + F])
        o += NM + F
    ident = misc[:, o:o + 128]

    # R holds the moving operands: PE row-group q serves features f = q mod 4;
    # partitions 32q+{0,1,2} hold (W[f], b[f], mask_token) of f = 4i+q at
    # free cols i*D.  The 3-partition transfers are slow, so they ride the
    # scalar-engine HWDGE ring (a separate FIFO - on the sync ring they would
    # stall the output chunks queued behind them) in two column waves: the
    # first wave unlocks the early output chunks, the tail streams in behind.
    R = singles.tile([128, NI * D], FMM, tag="R")
    for q in range(4):
        nc.scalar.dma_start(out=R[32 * q:32 * q + 3, 0:4 * D],
                            in_=r_ap[3 * q:3 * q + 3, 0:4 * D])
    for q in range(4):
        nc.scalar.dma_start(out=R[32 * q:32 * q + 3, 4 * D:],
                            in_=r_ap[3 * q:3 * q + 3, 4 * D:])

    # P (one per batch-tile, ping-pong) packs the per-batch stationary data in
    # natural layout so one PE transpose per 128 columns yields the stationary
    # tile: column 128i + 32g + c = component c (xs, om, m) of feature 4i+g.
    # Columns 32g+3..32g+31 are never written: they land in stationary rows
    # the K=3 matmuls never read, so no zeroing is needed.
    Ps = [singles.tile([128, NI * 128], F32, tag=f"P{t}", name=f"P{t}")
          for t in range(NT)]

    # Feature map: f = 4i + g -> stationary pack S_i, PE row-group g.
    iov = iota[:, :].rearrange("p (i g) -> p i g", g=4)

    for T in range(NT):
        xt = xts[T]
        mpt = mpts[T]
        P = Ps[T]
        Pv = P[:, :].rearrange("p (i g c) -> p i g c", g=4, c=32)
        xsv = Pv[:, :, :, 0]
        omv = Pv[:, :, :, 1]
        mv = Pv[:, :, :, 2]
        xv = xt.rearrange("p (i g) -> p i g", g=4)

        # m = one-hot of mask positions: compare iota against every index
        # column in one broadcast op, then max-reduce over the index axis.
        eq = eqpool.tile([128, F * NM], F32, tag="eq")
        eqv = eq[:, :].rearrange("p (i g j) -> p i g j", g=4, j=NM)
        iov4 = iov.to_broadcast([128, NI, 4, NM])
        mpv4 = dataclasses.replace(
            mpt, ap=[[mpt.ap[0][0], 128], [0, NI], [0, 4], [1, NM]])
        nc.vector.tensor_tensor(eqv, iov4, mpv4, op=AOT.is_equal)
        nc.vector.tensor_reduce(mv, eqv, axis=mybir.AxisListType.X,
                                op=AOT.max)
        nc.vector.tensor_scalar(omv, mv, -1.0, 1.0, AOT.mult, AOT.add)
        nc.vector.tensor_tensor(xsv, xv, omv, op=AOT.mult)

        # PE transpose pack i -> stationary tile S_i (evicted by DVE/ACT).
        Ss = [None] * NI

        def make_S(i):
            pst = tpsum.tile([128, 128], F32, tag="pst", name="pst")
            nc.tensor.transpose(pst[:], P[:, 128 * i:128 * (i + 1)], ident)
            S = spool.tile([128, 128], FMM, tag="S", name="S")
            if i % 2 == 0:
                nc.vector.tensor_copy(S[:], pst[:])
            else:
                nc.scalar.copy(S[:], pst[:])
            Ss[i] = S

        # 8 K=3 row-tiled matmuls per chunk.  Feature pairs share one 2-bank
        # PSUM tile so each eviction moves [128, 1024] in a single op (the
        # fixed per-op cost is paid half as often); DVE and ACT alternate.
        def do_chunk(ci):
            s0, nf = CHUNKS[ci]
            stg = stpool.tile([128, 8 * D], F32, tag="stg", name="stg")
            for k2 in range(nf // 2):
                po = opsum.tile([128, 2 * D], F32, tag="po", name="po")
                for h in range(2):
                    k = 2 * k2 + h
                    f = s0 + k
                    i, g = divmod(f, 4)
                    nc.tensor.matmul(po[:, bass.ts(h, D)],
                                     lhsT=Ss[i][32 * g:32 * g + 3, :],
                                     rhs=R[32 * g:32 * g + 3, bass.ts(i, D)],
                                     start=True, stop=True,
                                     tile_position=(32 * g, 0))
                dst = stg[:, 2 * k2 * D:(2 * k2 + 2) * D]
                if k2 % 2 == 0:
                    nc.vector.tensor_copy(dst, po[:])
                else:
                    nc.scalar.copy(dst, po[:])
            nc.sync.dma_start(
                out=out_ap[T * 128:(T + 1) * 128, s0:s0 + nf, :],
                in_=stg[:, 0:nf * D].rearrange("p (a b) -> p a b", b=D))

        for i in range(NI):
            make_S(i)
        for ci in range(len(CHUNKS)):
            do_chunk(ci)


def _build_program():
    nc = bacc.Bacc("TRN2", target_bir_lowering=False, debug=False,
                   enable_asserts=True, num_devices=NCORES)
    misc_d = nc.dram_tensor("misc_in", [128, MISC_W], F32,
                            kind="ExternalInput")
    r_d = nc.dram_tensor("r_in", [12, NI * D], FMM, kind="ExternalInput")
    out_d = nc.dram_tensor("out", [B_LOC, F, D], F32, kind="ExternalOutput")

    with tile.TileContext(nc) as tc:
        with ExitStack() as ctx:
            _body(ctx, tc, misc_d.ap(), r_d.ap(), out_d.ap())
    nc.compile()
    return nc


def get_nc():
    if "nc" not in _cache:
        _cache["nc"] = _build_program()
    return _cache["nc"]


def make_in_maps(x, mask_positions, W, b, mask_token):
    xf = np.ascontiguousarray(np.asarray(x, dtype=np.float32))
    mpf = np.ascontiguousarray(np.asarray(mask_positions).astype(np.float32))
    Wf = np.asarray(W, dtype=np.float32)
    bf = np.asarray(b, dtype=np.float32)
    mtf = np.asarray(mask_token, dtype=np.float32)

    R_host = np.zeros((12, NI * D), dtype=np.float32)
    for q in range(4):
        for i in range(NI):
            f = 4 * i + q
            R_host[3 * q + 0, i * D:(i + 1) * D] = Wf[f]
            R_host[3 * q + 1, i * D:(i + 1) * D] = bf[f]
            R_host[3 * q + 2, i * D:(i + 1) * D] = mtf
    ident = np.eye(128, dtype=np.float32)
    iota = np.broadcast_to(np.arange(F, dtype=np.float32), (128, F))

    maps = []
    for cc in range(NCORES):
        s = cc * B_LOC
        parts = [iota]
        for t in range(NT):
            parts.append(mpf[s + t * 128:s + (t + 1) * 128])
            parts.append(xf[s + t * 128:s + (t + 1) * 128])
        parts.append(ident)
        misc = np.ascontiguousarray(
            np.concatenate(parts, axis=1, dtype=np.float32))
        assert misc.shape == (128, MISC_W)
        maps.append({"misc_in": misc, "r_in": R_host})
    return maps


def kernel(x, mask_positions, W, b, mask_token):
    nc = get_nc()
    maps = make_in_maps(x, mask_positions, W, b, mask_token)
    res = run_bass_kernel_spmd(nc, maps, core_ids=list(range(NCORES)))
    return np.concatenate([res.results[cc]["out"] for cc in range(NCORES)],
                          axis=0)


# revision 64
# speedup vs baseline: 1.2405x; 1.0166x over previous
"""Trainium2 Bass kernel for FeatureEmbedding (embedding_lookup).

Computes, for inputs x[B,F], mask_positions[B,NM], W[F,D], b[F,D], mask_token[D]:

    tokens[bb, f, :] = x[bb, f] * W[f, :] + b[f, :]
    tokens[bb, p, :] = mask_token          for p in mask_positions[bb]

Strategy (data-parallel over 8 NeuronCores, 256 batch rows each):
  - On device, build a one-hot mask m[bb,f] = any_j(mask_positions[bb,j]==f)
    with iota/is_equal vector ops, then om = 1-m and xs = x*om.
  - For each feature f the output tile out[:, f, :] (128 batch rows x 512) is
    a rank-3 product computed on the TensorEngine:
        out = xs[:,f] (x) W[f,:] + om[:,f] (x) b[f,:] + m[:,f] (x) mask_token
    i.e. a K=3 matmul with stationary rows (xs_f, om_f, m_f) and moving rows
    (W[f], b[f], mask_token).  The stationary triples are produced by PE
    transposes of a packed tile; four features run concurrently in the four
    32-row PE tile groups (row tiling).  PSUM tiles are evicted to SBUF by
    the Vector and Scalar engines (split), then streamed to HBM in 2 MiB DMAs.
  - Matmul operands use float32r (same 4-byte layout as f32; fast PE path).

The output write (32 MiB/core, 256 MiB total) is the roofline term.
"""

import dataclasses
import os
from contextlib import ExitStack

import numpy as np

import concourse.bass as bass
import concourse.tile as tile
from concourse import bacc, mybir
from concourse.bass import _add_dep_helper
from concourse.bass_utils import run_bass_kernel_spmd

B, F, D, NM = 2048, 64, 512, 8
NCORES = 8
B_LOC = B // NCORES          # 256 batch rows per core
NT = B_LOC // 128            # 2 partition-tiles of 128 batch rows
NI = F // 4                  # 16 stationary packs (4 features each)

F32 = mybir.dt.float32
# Output chunking: feature spans per output DMA.
CHUNKS = [(8 * k, 8) for k in range(8)]
# Packed small-input width: iota | (mp, x) per batch-tile | identity.
MISC_W = F + NT * (NM + F) + 128
# float32r: same bits as f32, streams through the PE at 1 cycle/row
# (fp32 proper needs 4).  Flip to False if precision ever becomes an issue.
USE_F32R = os.environ.get("KERNEL_NO_F32R", "") == ""
FMM = mybir.dt.float32r if USE_F32R else mybir.dt.float32
AOT = mybir.AluOpType

_cache: dict = {}


def _body(ctx: ExitStack, tc: tile.TileContext, x_ap, r_ap, out_ap):
    nc = tc.nc
    singles = ctx.enter_context(tc.tile_pool(name="singles", bufs=1))
    eqpool = ctx.enter_context(tc.tile_pool(name="eqp", bufs=2))
    spool = ctx.enter_context(tc.tile_pool(name="spool", bufs=20))
    stpool = ctx.enter_context(tc.tile_pool(name="stage", bufs=4))
    tpsum = ctx.enter_context(tc.tile_pool(name="tpsum", bufs=2, space="PSUM"))
    opsum = ctx.enter_context(tc.tile_pool(name="opsum", bufs=3, space="PSUM"))

    # All the small inputs (iota, mask positions, x, identity) arrive
    # host-packed in one tensor so a single sync-ring DMA (one completion
    # wait) delivers everything the mask chain needs.
    misc = singles.tile([128, MISC_W], F32, tag="misc")
    nc.sync.dma_start(out=misc[:], in_=x_ap[:, :])
    iota = misc[:, 0:F]
    o = F
    mpts, xts = [], []
    for t in range(NT):
        mpts.append(misc[:, o:o + NM])
        xts.append(misc[:, o + NM:o + NM + F])
        o += NM + F
    ident = misc[:, o:o + 128]

    # R holds the moving operands: PE row-group q serves features f = q mod 4;
    # partitions 32q+{0,1,2} hold (W[f], b[f], mask_token) of f = 4i+q at
    # free cols i*D.  The 3-partition transfers are slow, so they ride the
    # scalar-engine HWDGE ring (a separate FIFO - on the sync ring they would
    # stall the output chunks queued behind them) in two column waves: the
    # first wave unlocks the early output chunks, the tail streams in behind.
    R = singles.tile([128, NI * D], FMM, tag="R")
    for q in range(4):
        nc.scalar.dma_start(out=R[32 * q:32 * q + 3, 0:4 * D],
                            in_=r_ap[3 * q:3 * q + 3, 0:4 * D])
    for q in range(4):
        nc.scalar.dma_start(out=R[32 * q:32 * q + 3, 4 * D:],
                            in_=r_ap[3 * q:3 * q + 3, 4 * D:])

    # P (one per batch-tile, ping-pong) packs the per-batch stationary data in
    # natural layout so one PE transpose per 128 columns yields the stationary
    # tile: column 128i + 32g + c = component c (xs, om, m) of feature 4i+g.
    # Columns 32g+3..32g+31 are never written: they land in stationary rows
    # the K=3 matmuls never read, so no zeroing is needed.
    Ps = [singles.tile([128, NI * 128], F32, tag=f"P{t}", name=f"P{t}")
          for t in range(NT)]

    # Feature map: f = 4i + g -> stationary pack S_i, PE row-group g.
    iov = iota[:, :].rearrange("p (i g) -> p i g", g=4)

    first_dma = [None]
    for T in range(NT):
        xt = xts[T]
        mpt = mpts[T]
        P = Ps[T]
        Pv = P[:, :].rearrange("p (i g c) -> p i g c", g=4, c=32)
        xsv = Pv[:, :, :, 0]
        omv = Pv[:, :, :, 1]
        mv = Pv[:, :, :, 2]
        xv = xt.rearrange("p (i g) -> p i g", g=4)

        # m = one-hot of mask positions: compare iota against every index
        # column in one broadcast op, then max-reduce over the index axis.
        eq = eqpool.tile([128, F * NM], F32, tag="eq")
        eqv = eq[:, :].rearrange("p (i g j) -> p i g j", g=4, j=NM)
        iov4 = iov.to_broadcast([128, NI, 4, NM])
        mpv4 = dataclasses.replace(
            mpt, ap=[[mpt.ap[0][0], 128], [0, NI], [0, 4], [1, NM]])
        eq_tt = nc.vector.tensor_tensor(eqv, iov4, mpv4, op=AOT.is_equal)
        if T > 0 and first_dma[0] is not None:
            # Keep the Vector engine clear for batch-tile 0's critical chain:
            # this tile's mask work can wait until the first chunk is out.
            _add_dep_helper(eq_tt.ins, first_dma[0].ins, sync=True,
                            reason="defer T1 mask until T0 chunk0 DMA")
        nc.vector.tensor_reduce(mv, eqv, axis=mybir.AxisListType.X,
                                op=AOT.max)
        nc.vector.tensor_scalar(omv, mv, -1.0, 1.0, AOT.mult, AOT.add)
        nc.vector.tensor_tensor(xsv, xv, omv, op=AOT.mult)

        # PE transpose pack i -> stationary tile S_i (evicted by DVE/ACT).
        Ss = [None] * NI

        def make_S(i):
            pst = tpsum.tile([128, 128], F32, tag="pst", name="pst")
            nc.tensor.transpose(pst[:], P[:, 128 * i:128 * (i + 1)], ident)
            S = spool.tile([128, 128], FMM, tag="S", name="S")
            if i % 2 == 0:
                nc.vector.tensor_copy(S[:], pst[:])
            else:
                nc.scalar.copy(S[:], pst[:])
            Ss[i] = S

        # 8 K=3 row-tiled matmuls per chunk.  Feature pairs share one 2-bank
        # PSUM tile so each eviction moves [128, 1024] in a single op (the
        # fixed per-op cost is paid half as often); DVE and ACT alternate.
        def do_chunk(ci):
            s0, nf = CHUNKS[ci]
            stg = stpool.tile([128, 8 * D], F32, tag="stg", name="stg")
            for k2 in range(nf // 2):
                po = opsum.tile([128, 2 * D], F32, tag="po", name="po")
                for h in range(2):
                    k = 2 * k2 + h
                    f = s0 + k
                    i, g = divmod(f, 4)
                    nc.tensor.matmul(po[:, bass.ts(h, D)],
                                     lhsT=Ss[i][32 * g:32 * g + 3, :],
                                     rhs=R[32 * g:32 * g + 3, bass.ts(i, D)],
                                     start=True, stop=True,
                                     tile_position=(32 * g, 0))
                dst = stg[:, 2 * k2 * D:(2 * k2 + 2) * D]
                if k2 % 2 == 0:
                    nc.vector.tensor_copy(dst, po[:])
                else:
                    nc.scalar.copy(dst, po[:])
            dma = nc.sync.dma_start(
                out=out_ap[T * 128:(T + 1) * 128, s0:s0 + nf, :],
                in_=stg[:, 0:nf * D].rearrange("p (a b) -> p a b", b=D))
            if T == 0 and ci == 0:
                first_dma[0] = dma

        # Chunk 0 is emitted right after its two stationaries: the PE then
        # runs its matmul pairs back-to-back instead of interleaving the
        # other fourteen transposes ahead of them.
        make_S(0)
        make_S(1)
        do_chunk(0)
        for i in range(2, NI):
            make_S(i)
        for ci in range(1, len(CHUNKS)):
            do_chunk(ci)


def _build_program():
    nc = bacc.Bacc("TRN2", target_bir_lowering=False, debug=False,
                   enable_asserts=True, num_devices=NCORES)
    misc_d = nc.dram_tensor("misc_in", [128, MISC_W], F32,
                            kind="ExternalInput")
    r_d = nc.dram_tensor("r_in", [12, NI * D], FMM, kind="ExternalInput")
    out_d = nc.dram_tensor("out", [B_LOC, F, D], F32, kind="ExternalOutput")

    with tile.TileContext(nc) as tc:
        with ExitStack() as ctx:
            _body(ctx, tc, misc_d.ap(), r_d.ap(), out_d.ap())
    nc.compile()
    return nc


def get_nc():
    if "nc" not in _cache:
        _cache["nc"] = _build_program()
    return _cache["nc"]


def make_in_maps(x, mask_positions, W, b, mask_token):
    xf = np.ascontiguousarray(np.asarray(x, dtype=np.float32))
    mpf = np.ascontiguousarray(np.asarray(mask_positions).astype(np.float32))
    Wf = np.asarray(W, dtype=np.float32)
    bf = np.asarray(b, dtype=np.float32)
    mtf = np.asarray(mask_token, dtype=np.float32)

    R_host = np.zeros((12, NI * D), dtype=np.float32)
    for q in range(4):
        for i in range(NI):
            f = 4 * i + q
            R_host[3 * q + 0, i * D:(i + 1) * D] = Wf[f]
            R_host[3 * q + 1, i * D:(i + 1) * D] = bf[f]
            R_host[3 * q + 2, i * D:(i + 1) * D] = mtf
    ident = np.eye(128, dtype=np.float32)
    iota = np.broadcast_to(np.arange(F, dtype=np.float32), (128, F))

    maps = []
    for cc in range(NCORES):
        s = cc * B_LOC
        parts = [iota]
        for t in range(NT):
            parts.append(mpf[s + t * 128:s + (t + 1) * 128])
            parts.append(xf[s + t * 128:s + (t + 1) * 128])
        parts.append(ident)
        misc = np.ascontiguousarray(
            np.concatenate(parts, axis=1, dtype=np.float32))
        assert misc.shape == (128, MISC_W)
        maps.append({"misc_in": misc, "r_in": R_host})
    return maps


def kernel(x, mask_positions, W, b, mask_token):
    nc = get_nc()
    maps = make_in_maps(x, mask_positions, W, b, mask_token)
    res = run_bass_kernel_spmd(nc, maps, core_ids=list(range(NCORES)))
    return np.concatenate([res.results[cc]["out"] for cc in range(NCORES)],
                          axis=0)
